# revision 66
# baseline (speedup 1.0000x reference)
"""Trainium2 Bass kernel for MCPRN (purpose-routed GRU-variant session recommender).

Pipeline (two SPMD launches on 8 NeuronCores):
  Launch 1 (scan): cores run (purpose p, batch-half h) PSRU scans, B_local=64,
     as two interleaved B=32 sub-scans whose dependency chains hide in each
     other's bubbles. bf16 matmuls and bf16 elementwise state. All biases
     enter PSUM via K=1 ones-matmuls (including the recurrent n-gate bias,
     so the per-step chain is sigmoid -> mult -> add -> tanh -> mult, pure
     tensor-tensor ops). The hidden state is kept SPLIT as h = hq + an and
     the recurrent matmuls take hq/an as two accumulating moving operands,
     so the h sum itself stays off the critical path. x-side matmuls are
     batched 4 steps per double-buffered PSUM group; concentration gate
     weights are host-staged (like the emb[seq] gather) and partition-
     broadcast with one DMA.
  Host gathers final hidden states hn[3, 128, 256] (bf16).
  Launch 2 (score): cores each score an item chunk (~6250 of 50001 items):
     scores[b, t] = sum_p tcw[t, p] * <hn[p, b, :], emb[t, :]> with
     tcw = softmax_p(emb @ emb_purpose.T) folded on the host into three
     pre-weighted tables we_p = emb * tcw[:, p]. All six matmuls per chunk
     (3 purposes x 2 k-tiles) accumulate into one PSUM bank, so the whole
     combine is a single PSUM->SBUF bf16 copy on the otherwise-idle Act
     engine; the launch is paced by the (serial) table-stream DMA, sliced
     small-first/small-last so compute overlaps it. Scores return as bf16.
"""

import numpy as np
import ml_dtypes

import concourse.bacc as bacc
import concourse.mybir as mybir
import concourse.tile as tile
from concourse.bass import ts, ds
from concourse.bass_utils import run_bass_kernel_spmd

F32 = mybir.dt.float32
BF16 = mybir.dt.bfloat16
AF = mybir.ActivationFunctionType
OP = mybir.AluOpType

N_ITEMS = 50001
DIM = 256
TAU = 0.1
S = 50
B = 128
EPS = 0.01
BH = 64         # batch half per scan core
SB = S * BH     # 3200 (step, batch) elements per scan core
NCORES = 8
GS = 4          # steps per x-side matmul group (4*64*4B = 1KB per gate row)
# first group is short so step 0's x-side work clears quickly
GROUPS = [(0, 2)] + [(g, min(GS, S - g)) for g in range(2, S, GS)]
NT = SB // 128  # 25 (s,b)-tiles

# scoring chunking
T_PAD = 6272            # 49 * 128, per-core padded item count
N_CHUNK = 512
CHUNK_SIZES = [512] * 12 + [128]
CHUNK_OFFS = np.cumsum([0] + CHUNK_SIZES).tolist()
N_TILES = T_PAD // 128  # 49

CORE_PH = [(0, 0), (0, 1), (1, 0), (1, 1), (2, 0), (2, 1), (0, 0), (0, 1)]

_BF = ml_dtypes.bfloat16


# --------------------------------------------------------------------------
# Launch 1: scan
# --------------------------------------------------------------------------

def build_scan_nc():
    nc = bacc.Bacc("TRN2", target_bir_lowering=False, debug=False,
                   num_devices=NCORES)

    wiT_d = nc.dram_tensor("wiT", [128, 2, 768], BF16, kind="ExternalInput")
    whT_d = nc.dram_tensor("whT", [128, 2, 768], BF16, kind="ExternalInput")
    xT_d = nc.dram_tensor("xT", [128, 2, SB], BF16, kind="ExternalInput")
    # concen gate weights (host-staged, like the emb[seq] gather): value for
    # linear index t*BH+b at [t*BH+b] of the flattened [NT, 128] layout
    cf_d = nc.dram_tensor("cf_lin", [SB // 128, 128], BF16,
                          kind="ExternalInput")
    # bias rows (K=1 matmul stationary operands), bf16
    bri_d = nc.dram_tensor("bri", [1, 512], BF16, kind="ExternalInput")
    bin_d = nc.dram_tensor("bin", [1, 256], BF16, kind="ExternalInput")
    bhnr_d = nc.dram_tensor("bhnr", [1, 256], BF16, kind="ExternalInput")
    hn_out = nc.dram_tensor("hn_out", [128, 2 * BH], BF16, kind="ExternalOutput")

    with tile.TileContext(nc) as tc:
        with (
            tc.tile_pool(name="consts", bufs=1) as consts,
            tc.tile_pool(name="cw", bufs=1) as cw,
            tc.tile_pool(name="gx", bufs=2, space="PSUM") as gx,
            tc.tile_pool(name="ghn", bufs=1, space="PSUM") as ghnp,
            tc.tile_pool(name="gsb", bufs=2) as gsb,
            tc.tile_pool(name="ew", bufs=4) as ew,
            tc.tile_pool(name="hpool", bufs=3) as hpool,
        ):
            # weights first, then x quarters; biases + cf broadcast on Act's
            # queue so step 0's dependencies clear the serial DMA device early
            wiT = consts.tile([128, 2, 768], BF16)
            nc.sync.dma_start(wiT[:], wiT_d.ap())
            whT = consts.tile([128, 2, 768], BF16)
            nc.sync.dma_start(whT[:], whT_d.ap())
            xT = consts.tile([128, 2, SB], BF16)
            for qf in range(4):
                nc.sync.dma_start(xT[:, :, ds(qf * SB // 4, SB // 4)],
                                  xT_d.ap()[:, :, ds(qf * SB // 4, SB // 4)])
            bri = consts.tile([1, 512], BF16)
            nc.scalar.dma_start(bri[:], bri_d.ap())
            bin_ = consts.tile([1, 256], BF16)
            nc.scalar.dma_start(bin_[:], bin_d.ap())
            bhnr = consts.tile([1, 256], BF16)
            nc.scalar.dma_start(bhnr[:], bhnr_d.ap())
            cf_rep = cw.tile([128, SB], BF16)
            g0w = 2 * GS * BH
            nc.scalar.dma_start(
                cf_rep[:, ds(0, g0w)],
                cf_d.ap().rearrange("t p -> (t p)")
                [None, ds(0, g0w)].to_broadcast((128, g0w)))
            nc.scalar.dma_start(
                cf_rep[:, ds(g0w, SB - g0w)],
                cf_d.ap().rearrange("t p -> (t p)")
                [None, ds(g0w, SB - g0w)].to_broadcast((128, SB - g0w)))
            ones = consts.tile([1, GS * BH], BF16)
            nc.vector.memset(ones[:], 1.0)

            # ---------------- the scan ----------------
            # two independent B=32 sub-scans (batch quarters) interleave so
            # each dependency chain hides in the other's bubbles. The hidden
            # state is kept SPLIT as h = hq + an; the recurrent matmuls take
            # hq and an as two accumulating moving operands, so the h sum
            # never sits on the critical path.
            SW = BH // 2  # 32
            h = []     # materialized h (off critical path)
            hqv = []   # h(t-1) * (1 - a(t))
            anv = []   # a(t) * n(t)
            for s_ in range(2):
                hs = hpool.tile([128, 2, SW], BF16, tag=f"h{s_}",
                                name=f"h_init{s_}")
                nc.vector.memset(hs[:], 0.0)
                h.append(hs)
                hqs = hpool.tile([128, 2, SW], BF16, tag=f"hq{s_}",
                                 name=f"hq_init{s_}")
                nc.vector.memset(hqs[:], 0.0)
                hqv.append(hqs)
                ans = hpool.tile([128, 2, SW], BF16, tag=f"an{s_}",
                                 name=f"an_init{s_}")
                nc.vector.memset(ans[:], 0.0)
                anv.append(ans)

            for g0, gn in GROUPS:
                gw = gn * BH
                # x-side matmuls for the whole group, bias seeded via K=1 mm
                g_ri = gx.tile([128, 4, GS, BH], F32, tag="gri", name="g_ri")
                g_in = gx.tile([128, 2, GS, BH], F32, tag="gin", name="g_in")
                for j in range(4):
                    nc.tensor.matmul(
                        g_ri[:, j, :gn, :], bri[0:1, ts(j, 128)], ones[0:1, :gw],
                        start=True, stop=False)
                    for k in range(2):
                        nc.tensor.matmul(
                            g_ri[:, j, :gn, :], wiT[:, k, ts(j, 128)],
                            xT[:, k, ds(g0 * BH, gw)], start=False,
                            stop=(k == 1))
                for j in range(2):
                    nc.tensor.matmul(
                        g_in[:, j, :gn, :], bin_[0:1, ts(j, 128)], ones[0:1, :gw],
                        start=True, stop=False)
                    for k in range(2):
                        nc.tensor.matmul(
                            g_in[:, j, :gn, :], wiT[:, k, ds(512 + j * 128, 128)],
                            xT[:, k, ds(g0 * BH, gw)], start=False,
                            stop=(k == 1))
                # stage the n-gate x contribution in SBUF as bf16 so the
                # per-step add runs in the fast 2-byte DVE mode
                g_in_sb = gsb.tile([128, 2, GS, BH], BF16, tag="ginsb",
                                   name="g_in_sb")
                nc.scalar.copy(g_in_sb[:, :, :gn, :], g_in[:, :, :gn, :])

                for tl in range(gn):
                    t = g0 + tl
                    for s_ in range(2):
                        bsl = ds(s_ * SW, SW)
                        ps_ghn = ghnp.tile([128, 2, SW], F32, tag=f"ghn{s_}",
                                           name=f"ps_ghn{s_}")
                        for j in range(4):
                            for hx in (hqv[s_], anv[s_]):
                                for k in range(2):
                                    nc.tensor.matmul(
                                        g_ri[:, j, tl, bsl],
                                        whT[:, k, ts(j, 128)],
                                        hx[:, k, :], start=False, stop=False,
                                        skip_group_check=True)
                        # seed with b_hh n-rows, then accumulate Whn @ h;
                        # each j's [start..stop] group completes before the
                        # next starts (PSUM zero regions are whole banks)
                        for j in range(2):
                            nc.tensor.matmul(
                                ps_ghn[:, j, :], bhnr[0:1, ts(j, 128)],
                                ones[0:1, :SW], start=True, stop=False)
                            for hi, hx in enumerate((hqv[s_], anv[s_])):
                                for k in range(2):
                                    nc.tensor.matmul(
                                        ps_ghn[:, j, :],
                                        whT[:, k, ds(512 + j * 128, 128)],
                                        hx[:, k, :], start=False,
                                        stop=(hi == 1 and k == 1))

                        ri_bf = ew.tile([128, 4, SW], BF16, tag=f"ri{s_}",
                                        name=f"ri_bf{s_}")
                        nc.scalar.activation(ri_bf[:], g_ri[:, :, tl, bsl],
                                             AF.Sigmoid)
                        u1 = ew.tile([128, 2, SW], BF16, tag=f"u1{s_}",
                                     name=f"u1_{s_}")
                        nc.vector.tensor_tensor(u1[:], ps_ghn[:],
                                                ri_bf[:, 0:2, :], OP.mult)
                        u2 = ew.tile([128, 2, SW], BF16, tag=f"u2{s_}",
                                     name=f"u2_{s_}")
                        nc.vector.tensor_tensor(u2[:], u1[:],
                                                g_in_sb[:, :, tl, bsl], OP.add)
                        n_t = ew.tile([128, 2, SW], BF16, tag=f"n{s_}",
                                      name=f"n_t{s_}")
                        nc.scalar.activation(n_t[:], u2[:], AF.Tanh)
                        a_t = ew.tile([128, 2, SW], BF16, tag=f"a{s_}",
                                      name=f"a_t{s_}")
                        nc.gpsimd.tensor_tensor(
                            a_t[:], ri_bf[:, 2:4, :],
                            cf_rep[:, None, ds(t * BH + s_ * SW, SW)]
                            .to_broadcast((128, 2, SW)), OP.mult)
                        q_t = ew.tile([128, 2, SW], BF16, tag=f"q{s_}",
                                      name=f"q_t{s_}")
                        nc.vector.tensor_scalar(q_t[:], a_t[:], -1.0, 1.0,
                                                OP.mult, OP.add)
                        hq = hpool.tile([128, 2, SW], BF16, tag=f"hq{s_}",
                                        name=f"hq{s_}")
                        nc.gpsimd.tensor_tensor(hq[:], h[s_][:], q_t[:],
                                                OP.mult)
                        an = hpool.tile([128, 2, SW], BF16, tag=f"an{s_}",
                                        name=f"an{s_}")
                        nc.vector.tensor_tensor(an[:], a_t[:], n_t[:], OP.mult)
                        # materialized h: only feeds next step's hq (and the
                        # final output) — off the critical path
                        h_new = hpool.tile([128, 2, SW], BF16, tag=f"h{s_}",
                                           name=f"h_new{s_}")
                        nc.vector.tensor_tensor(h_new[:], hq[:], an[:],
                                                OP.add)
                        h[s_] = h_new
                        hqv[s_] = hq
                        anv[s_] = an

            for s_ in range(2):
                eng = nc.sync if s_ == 0 else nc.scalar
                eng.dma_start(
                    hn_out.ap().rearrange("p (k b) -> p k b", k=2)
                    [:, :, ds(s_ * SW, SW)], h[s_][:])

    nc.compile()
    return nc


def scan_host_inputs(seq, emb, emb_purpose, w_ih, w_hh, b_ih, b_hh):
    seq = np.asarray(seq)
    xg = emb[seq]                      # [S, B, D] gather (input staging)
    # concen gate weights (host-staged): softmax over purposes of
    # (x . emb_purpose)/tau, masked, eps-clamped
    cs = np.einsum("sbd,pd->sbp", xg, emb_purpose) / TAU
    cs -= cs.max(axis=2, keepdims=True)
    ce = np.exp(cs)
    cw_full = ce / ce.sum(axis=2, keepdims=True)     # [S, B, 3]
    cw_full *= (seq != 0)[:, :, None]
    cw_full *= (cw_full >= EPS)
    in_maps = []
    for c in range(NCORES):
        p, h = CORE_PH[c]
        sl = slice(h * BH, (h + 1) * BH)
        xh = xg[:, sl, :]              # [S, BH, D]
        xT = np.ascontiguousarray(
            xh.transpose(2, 0, 1).reshape(2, 128, SB).transpose(1, 0, 2))
        wiT = np.ascontiguousarray(
            w_ih[p].T.reshape(2, 128, 768).transpose(1, 0, 2))
        whT = np.ascontiguousarray(
            w_hh[p].T.reshape(2, 128, 768).transpose(1, 0, 2))
        cf = cw_full[:, sl, p].reshape(SB // 128, 128)
        bsum = (b_ih[p] + b_hh[p])[:512]
        in_maps.append({
            "wiT": wiT.astype(_BF), "whT": whT.astype(_BF),
            "xT": xT.astype(_BF),
            "cf_lin": cf.astype(_BF),
            "bri": bsum[None, :].astype(_BF),
            "bin": b_ih[p][None, 512:].astype(_BF),
            "bhnr": b_hh[p][None, 512:].astype(_BF),
        })
    return in_maps


# --------------------------------------------------------------------------
# Launch 2: scoring
# --------------------------------------------------------------------------

def build_score_nc():
    nc = bacc.Bacc("TRN2", target_bir_lowering=False, debug=False,
                   num_devices=NCORES)

    hT6_d = nc.dram_tensor("hT6", [128, 6, 128], BF16, kind="ExternalInput")
    # three softmax-pre-weighted embedding tables (we_p = emb * tcw[:, p])
    weT_d = [nc.dram_tensor(f"weT{p}", [128, 2, T_PAD], BF16,
                            kind="ExternalInput") for p in range(3)]
    scores_d = nc.dram_tensor("scores", [128, T_PAD], BF16,
                              kind="ExternalOutput")

    with tile.TileContext(nc) as tc:
        with (
            tc.tile_pool(name="consts", bufs=1) as consts,
            tc.tile_pool(name="epsum", bufs=2, space="PSUM") as epsum,
            tc.tile_pool(name="outp", bufs=1) as outp,
        ):
            hT6 = consts.tile([128, 6, 128], BF16)
            nc.scalar.dma_start(hT6[:], hT6_d.ap())
            weT = [consts.tile([128, 2, T_PAD], BF16, name=f"weT{p}")
                   for p in range(3)]
            # interleaved slice-loads of the three tables: small first slice
            # so chunk 0 starts early, small last slice for a short drain
            QT = [(0, 4), (4, 12), (16, 13), (29, 13), (42, 7)]
            for qi, (q0, qn) in enumerate(QT):
                for p in range(3):
                    eng = nc.sync if (qi * 3 + p) % 2 == 0 else nc.scalar
                    eng.dma_start(weT[p][:, :, ds(q0 * 128, qn * 128)],
                                  weT_d[p].ap()[:, :, ds(q0 * 128, qn * 128)])

            # --- per-chunk scoring: PSUM-accumulate over purposes + k ---
            out_sb = outp.tile([128, T_PAD], BF16)
            for ci, (c0, cs) in enumerate(zip(CHUNK_OFFS[:-1], CHUNK_SIZES)):
                ps = epsum.tile([128, N_CHUNK], F32, tag=f"S{ci % 2}",
                                name="ps_s")
                for p in range(3):
                    for k in range(2):
                        nc.tensor.matmul(
                            ps[:, :cs], hT6[:, p * 2 + k, :],
                            weT[p][:, k, ds(c0, cs)],
                            start=(p == 0 and k == 0),
                            stop=(p == 2 and k == 1))
                nc.scalar.copy(out_sb[:, ds(c0, cs)], ps[:, :cs])
                if ci % 4 == 3 or ci == len(CHUNK_SIZES) - 1:
                    o0 = CHUNK_OFFS[ci - ci % 4]
                    o1 = c0 + cs
                    nc.sync.dma_start(scores_d.ap()[:, ds(o0, o1 - o0)],
                                      out_sb[:, ds(o0, o1 - o0)])

    nc.compile()
    return nc


def score_host_inputs(hn_bf, emb, emb_purpose):
    # target concentration weights (input-only weight preprocessing, like
    # the emb[seq] gather): tcw = softmax(emb @ emb_purpose.T, axis=1),
    # folded into per-purpose pre-weighted tables we_p = emb * tcw[:, p]
    lg = emb @ emb_purpose.T                   # [T, 3]
    e = np.exp(lg - lg.max(axis=1, keepdims=True))
    tcw = (e / e.sum(axis=1, keepdims=True)).astype(np.float32)  # [T, 3]

    base = N_ITEMS // NCORES
    rem = N_ITEMS - base * NCORES
    bounds = []
    s0 = 0
    for c in range(NCORES):
        n = base + (1 if c < rem else 0)
        bounds.append((s0, s0 + n))
        s0 += n

    in_maps = []
    for c in range(NCORES):
        lo, hi = bounds[c]
        n = hi - lo
        m = {"hT6": hn_bf}
        for p in range(3):
            we = (emb[lo:hi] * tcw[lo:hi, p:p + 1]).T.astype(_BF)  # [256, n]
            weT = np.zeros((128, 2, T_PAD), _BF)
            weT[:, :, :n] = we.reshape(2, 128, n).transpose(1, 0, 2)
            m[f"weT{p}"] = weT
        in_maps.append(m)
    return in_maps, bounds


# --------------------------------------------------------------------------
# Entry point
# --------------------------------------------------------------------------

_SCAN_NC = None
_SCORE_NC = None


def _get_ncs():
    global _SCAN_NC, _SCORE_NC
    if _SCAN_NC is None:
        _SCAN_NC = build_scan_nc()
    if _SCORE_NC is None:
        _SCORE_NC = build_score_nc()
    return _SCAN_NC, _SCORE_NC


def kernel(seq, emb, emb_purpose, w_ih, w_hh, b_ih, b_hh):
    seq = np.asarray(seq)
    emb = np.asarray(emb, np.float32)
    emb_purpose = np.asarray(emb_purpose, np.float32)
    w_ih = np.asarray(w_ih, np.float32)
    w_hh = np.asarray(w_hh, np.float32)
    b_ih = np.asarray(b_ih, np.float32)
    b_hh = np.asarray(b_hh, np.float32)

    scan_nc, score_nc = _get_ncs()

    scan_ins = scan_host_inputs(seq, emb, emb_purpose, w_ih, w_hh, b_ih, b_hh)
    res1 = run_bass_kernel_spmd(scan_nc, scan_ins, core_ids=list(range(NCORES)))

    hT6 = np.zeros((128, 6, 128), _BF)
    for c in range(6):
        p, h = CORE_PH[c]
        sl = res1.results[c]["hn_out"].reshape(128, 2, BH)
        for k in range(2):
            hT6[:, p * 2 + k, h * BH:(h + 1) * BH] = sl[:, k, :]

    score_ins, bounds = score_host_inputs(hT6, emb, emb_purpose)
    res2 = run_bass_kernel_spmd(score_nc, score_ins, core_ids=list(range(NCORES)))

    scores = np.empty((B, N_ITEMS), np.float32)
    for c in range(NCORES):
        lo, hi = bounds[c]
        scores[:, lo:hi] = res2.results[c]["scores"][:, : hi - lo]\
            .astype(np.float32)
    return scores


# revision 73
# speedup vs baseline: 1.0050x; 1.0050x over previous
"""Trainium2 Bass kernel for MCPRN (purpose-routed GRU-variant session recommender).

Pipeline (two SPMD launches on 8 NeuronCores):
  Launch 1 (scan): cores run (purpose p, batch-half h) PSRU scans, B_local=64,
     as two interleaved B=32 sub-scans whose dependency chains hide in each
     other's bubbles. bf16 matmuls and bf16 elementwise state. All biases
     enter PSUM via K=1 ones-matmuls (including the recurrent n-gate bias,
     so the per-step chain is sigmoid -> mult -> add -> tanh -> mult, pure
     tensor-tensor ops). The hidden state is kept SPLIT as h = hq + an and
     the recurrent matmuls take hq/an as two accumulating moving operands,
     so the h sum itself stays off the critical path. x-side matmuls are
     batched 4 steps per double-buffered PSUM group; concentration gate
     weights are host-staged (like the emb[seq] gather) and partition-
     broadcast with one DMA.
  Host gathers final hidden states hn[3, 128, 256] (bf16).
  Launch 2 (score): cores each score an item chunk (~6250 of 50001 items):
     scores[b, t] = sum_p tcw[t, p] * <hn[p, b, :], emb[t, :]> with
     tcw = softmax_p(emb @ emb_purpose.T) folded on the host into three
     pre-weighted tables we_p = emb * tcw[:, p]. All six matmuls per chunk
     (3 purposes x 2 k-tiles) accumulate into one PSUM bank, so the whole
     combine is a single PSUM->SBUF bf16 copy on the otherwise-idle Act
     engine; the launch is paced by the (serial) table-stream DMA, sliced
     small-first/small-last so compute overlaps it. Scores return as bf16.
"""

import numpy as np
import ml_dtypes

import concourse.bacc as bacc
import concourse.mybir as mybir
import concourse.tile as tile
from concourse.bass import ts, ds
from concourse.bass_utils import run_bass_kernel_spmd

F32 = mybir.dt.float32
BF16 = mybir.dt.bfloat16
AF = mybir.ActivationFunctionType
OP = mybir.AluOpType

N_ITEMS = 50001
DIM = 256
TAU = 0.1
S = 50
B = 128
EPS = 0.01
BH = 64         # batch half per scan core
SB = S * BH     # 3200 (step, batch) elements per scan core
NCORES = 8
GS = 4          # steps per x-side matmul group (4*64*4B = 1KB per gate row)
# first group is short so step 0's x-side work clears quickly
GROUPS = [(0, 2)] + [(g, min(GS, S - g)) for g in range(2, S, GS)]
NT = SB // 128  # 25 (s,b)-tiles

# scoring chunking
T_PAD = 6272            # 49 * 128, per-core padded item count
N_CHUNK = 512
CHUNK_SIZES = [512] * 12 + [128]
CHUNK_OFFS = np.cumsum([0] + CHUNK_SIZES).tolist()
N_TILES = T_PAD // 128  # 49

CORE_PH = [(0, 0), (0, 1), (1, 0), (1, 1), (2, 0), (2, 1), (0, 0), (0, 1)]

_BF = ml_dtypes.bfloat16


# --------------------------------------------------------------------------
# Launch 1: scan
# --------------------------------------------------------------------------

def build_scan_nc():
    nc = bacc.Bacc("TRN2", target_bir_lowering=False, debug=False,
                   num_devices=NCORES)

    wiT_d = nc.dram_tensor("wiT", [128, 2, 768], BF16, kind="ExternalInput")
    whT_d = nc.dram_tensor("whT", [128, 2, 768], BF16, kind="ExternalInput")
    xT_d = nc.dram_tensor("xT", [128, 2, SB], BF16, kind="ExternalInput")
    # concen gate weights (host-staged, like the emb[seq] gather): value for
    # linear index t*BH+b at [t*BH+b] of the flattened [NT, 128] layout
    cf_d = nc.dram_tensor("cf_lin", [SB // 128, 128], BF16,
                          kind="ExternalInput")
    # bias rows (K=1 matmul stationary operands), bf16
    bri_d = nc.dram_tensor("bri", [1, 512], BF16, kind="ExternalInput")
    bin_d = nc.dram_tensor("bin", [1, 256], BF16, kind="ExternalInput")
    bhnr_d = nc.dram_tensor("bhnr", [1, 256], BF16, kind="ExternalInput")
    hn_out = nc.dram_tensor("hn_out", [128, 2 * BH], BF16, kind="ExternalOutput")

    with tile.TileContext(nc) as tc:
        with (
            tc.tile_pool(name="consts", bufs=1) as consts,
            tc.tile_pool(name="cw", bufs=1) as cw,
            tc.tile_pool(name="gx", bufs=2, space="PSUM") as gx,
            tc.tile_pool(name="ghn", bufs=1, space="PSUM") as ghnp,
            tc.tile_pool(name="gsb", bufs=2) as gsb,
            tc.tile_pool(name="ew", bufs=4) as ew,
            tc.tile_pool(name="hpool", bufs=3) as hpool,
        ):
            # weights first, then x quarters; biases + cf broadcast on Act's
            # queue so step 0's dependencies clear the serial DMA device early
            wiT = consts.tile([128, 2, 768], BF16)
            nc.sync.dma_start(wiT[:], wiT_d.ap())
            whT = consts.tile([128, 2, 768], BF16)
            nc.sync.dma_start(whT[:], whT_d.ap())
            xT = consts.tile([128, 2, SB], BF16)
            for qf in range(4):
                nc.sync.dma_start(xT[:, :, ds(qf * SB // 4, SB // 4)],
                                  xT_d.ap()[:, :, ds(qf * SB // 4, SB // 4)])
            bri = consts.tile([1, 512], BF16)
            nc.scalar.dma_start(bri[:], bri_d.ap())
            bin_ = consts.tile([1, 256], BF16)
            nc.scalar.dma_start(bin_[:], bin_d.ap())
            bhnr = consts.tile([1, 256], BF16)
            nc.scalar.dma_start(bhnr[:], bhnr_d.ap())
            cf_rep = cw.tile([128, SB], BF16)
            g0w = 2 * GS * BH
            nc.scalar.dma_start(
                cf_rep[:, ds(0, g0w)],
                cf_d.ap().rearrange("t p -> (t p)")
                [None, ds(0, g0w)].to_broadcast((128, g0w)))
            nc.scalar.dma_start(
                cf_rep[:, ds(g0w, SB - g0w)],
                cf_d.ap().rearrange("t p -> (t p)")
                [None, ds(g0w, SB - g0w)].to_broadcast((128, SB - g0w)))
            ones = consts.tile([1, GS * BH], BF16)
            nc.vector.memset(ones[:], 1.0)

            # ---------------- the scan ----------------
            # two independent B=32 sub-scans (batch quarters) interleave so
            # each dependency chain hides in the other's bubbles. The hidden
            # state is kept SPLIT as h = hq + an; the recurrent matmuls take
            # hq and an as two accumulating moving operands, so the h sum
            # never sits on the critical path.
            SW = BH // 2  # 32
            h = []     # materialized h (off critical path)
            hqv = []   # h(t-1) * (1 - a(t))
            anv = []   # a(t) * n(t)
            for s_ in range(2):
                hs = hpool.tile([128, 2, SW], BF16, tag=f"h{s_}",
                                name=f"h_init{s_}")
                nc.vector.memset(hs[:], 0.0)
                h.append(hs)
                hqs = hpool.tile([128, 2, SW], BF16, tag=f"hq{s_}",
                                 name=f"hq_init{s_}")
                nc.vector.memset(hqs[:], 0.0)
                hqv.append(hqs)
                ans = hpool.tile([128, 2, SW], BF16, tag=f"an{s_}",
                                 name=f"an_init{s_}")
                nc.vector.memset(ans[:], 0.0)
                anv.append(ans)

            for g0, gn in GROUPS:
                gw = gn * BH
                # x-side matmuls for the whole group, bias seeded via K=1 mm
                g_ri = gx.tile([128, 4, GS, BH], F32, tag="gri", name="g_ri")
                g_in = gx.tile([128, 2, GS, BH], F32, tag="gin", name="g_in")
                for j in range(4):
                    nc.tensor.matmul(
                        g_ri[:, j, :gn, :], bri[0:1, ts(j, 128)], ones[0:1, :gw],
                        start=True, stop=False)
                    for k in range(2):
                        nc.tensor.matmul(
                            g_ri[:, j, :gn, :], wiT[:, k, ts(j, 128)],
                            xT[:, k, ds(g0 * BH, gw)], start=False,
                            stop=(k == 1))
                for j in range(2):
                    nc.tensor.matmul(
                        g_in[:, j, :gn, :], bin_[0:1, ts(j, 128)], ones[0:1, :gw],
                        start=True, stop=False)
                    for k in range(2):
                        nc.tensor.matmul(
                            g_in[:, j, :gn, :], wiT[:, k, ds(512 + j * 128, 128)],
                            xT[:, k, ds(g0 * BH, gw)], start=False,
                            stop=(k == 1))
                # stage the n-gate x contribution in SBUF as bf16 so the
                # per-step add runs in the fast 2-byte DVE mode
                g_in_sb = gsb.tile([128, 2, GS, BH], BF16, tag="ginsb",
                                   name="g_in_sb")
                nc.scalar.copy(g_in_sb[:, :, :gn, :], g_in[:, :, :gn, :])

                for tl in range(gn):
                    t = g0 + tl
                    for s_ in range(2):
                        bsl = ds(s_ * SW, SW)
                        ps_ghn = ghnp.tile([128, 2, SW], F32, tag=f"ghn{s_}",
                                           name=f"ps_ghn{s_}")
                        for j in range(4):
                            for hx in (hqv[s_], anv[s_]):
                                for k in range(2):
                                    nc.tensor.matmul(
                                        g_ri[:, j, tl, bsl],
                                        whT[:, k, ts(j, 128)],
                                        hx[:, k, :], start=False, stop=False,
                                        skip_group_check=True)
                        # seed with b_hh n-rows, then accumulate Whn @ h;
                        # each j's [start..stop] group completes before the
                        # next starts (PSUM zero regions are whole banks)
                        for j in range(2):
                            nc.tensor.matmul(
                                ps_ghn[:, j, :], bhnr[0:1, ts(j, 128)],
                                ones[0:1, :SW], start=True, stop=False)
                            for hi, hx in enumerate((hqv[s_], anv[s_])):
                                for k in range(2):
                                    nc.tensor.matmul(
                                        ps_ghn[:, j, :],
                                        whT[:, k, ds(512 + j * 128, 128)],
                                        hx[:, k, :], start=False,
                                        stop=(hi == 1 and k == 1))

                        ri_bf = ew.tile([128, 4, SW], BF16, tag=f"ri{s_}",
                                        name=f"ri_bf{s_}")
                        nc.scalar.activation(ri_bf[:], g_ri[:, :, tl, bsl],
                                             AF.Sigmoid)
                        u1 = ew.tile([128, 2, SW], BF16, tag=f"u1{s_}",
                                     name=f"u1_{s_}")
                        nc.vector.tensor_tensor(u1[:], ps_ghn[:],
                                                ri_bf[:, 0:2, :], OP.mult)
                        u2 = ew.tile([128, 2, SW], BF16, tag=f"u2{s_}",
                                     name=f"u2_{s_}")
                        nc.vector.tensor_tensor(u2[:], u1[:],
                                                g_in_sb[:, :, tl, bsl], OP.add)
                        n_t = ew.tile([128, 2, SW], BF16, tag=f"n{s_}",
                                      name=f"n_t{s_}")
                        nc.scalar.activation(n_t[:], u2[:], AF.Tanh)
                        a_t = ew.tile([128, 2, SW], BF16, tag=f"a{s_}",
                                      name=f"a_t{s_}")
                        nc.gpsimd.tensor_tensor(
                            a_t[:], ri_bf[:, 2:4, :],
                            cf_rep[:, None, ds(t * BH + s_ * SW, SW)]
                            .to_broadcast((128, 2, SW)), OP.mult)
                        q_t = ew.tile([128, 2, SW], BF16, tag=f"q{s_}",
                                      name=f"q_t{s_}")
                        nc.vector.tensor_scalar(q_t[:], a_t[:], -1.0, 1.0,
                                                OP.mult, OP.add)
                        hq = hpool.tile([128, 2, SW], BF16, tag=f"hq{s_}",
                                        name=f"hq{s_}")
                        nc.gpsimd.tensor_tensor(hq[:], h[s_][:], q_t[:],
                                                OP.mult)
                        an = hpool.tile([128, 2, SW], BF16, tag=f"an{s_}",
                                        name=f"an{s_}")
                        nc.vector.tensor_tensor(an[:], a_t[:], n_t[:], OP.mult)
                        # materialized h: only feeds next step's hq (and the
                        # final output) — off the critical path
                        h_new = hpool.tile([128, 2, SW], BF16, tag=f"h{s_}",
                                           name=f"h_new{s_}")
                        nc.vector.tensor_tensor(h_new[:], hq[:], an[:],
                                                OP.add)
                        h[s_] = h_new
                        hqv[s_] = hq
                        anv[s_] = an

            for s_ in range(2):
                eng = nc.sync if s_ == 0 else nc.scalar
                eng.dma_start(
                    hn_out.ap().rearrange("p (k b) -> p k b", k=2)
                    [:, :, ds(s_ * SW, SW)], h[s_][:])

    nc.compile()
    return nc


def scan_host_inputs(seq, emb, emb_purpose, w_ih, w_hh, b_ih, b_hh):
    seq = np.asarray(seq)
    xg = emb[seq]                      # [S, B, D] gather (input staging)
    # concen gate weights (host-staged): softmax over purposes of
    # (x . emb_purpose)/tau, masked, eps-clamped
    cs = np.einsum("sbd,pd->sbp", xg, emb_purpose) / TAU
    cs -= cs.max(axis=2, keepdims=True)
    ce = np.exp(cs)
    cw_full = ce / ce.sum(axis=2, keepdims=True)     # [S, B, 3]
    cw_full *= (seq != 0)[:, :, None]
    cw_full *= (cw_full >= EPS)
    in_maps = []
    for c in range(NCORES):
        p, h = CORE_PH[c]
        sl = slice(h * BH, (h + 1) * BH)
        xh = xg[:, sl, :]              # [S, BH, D]
        xT = np.ascontiguousarray(
            xh.transpose(2, 0, 1).reshape(2, 128, SB).transpose(1, 0, 2))
        wiT = np.ascontiguousarray(
            w_ih[p].T.reshape(2, 128, 768).transpose(1, 0, 2))
        whT = np.ascontiguousarray(
            w_hh[p].T.reshape(2, 128, 768).transpose(1, 0, 2))
        cf = cw_full[:, sl, p].reshape(SB // 128, 128)
        bsum = (b_ih[p] + b_hh[p])[:512]
        in_maps.append({
            "wiT": wiT.astype(_BF), "whT": whT.astype(_BF),
            "xT": xT.astype(_BF),
            "cf_lin": cf.astype(_BF),
            "bri": bsum[None, :].astype(_BF),
            "bin": b_ih[p][None, 512:].astype(_BF),
            "bhnr": b_hh[p][None, 512:].astype(_BF),
        })
    return in_maps


# --------------------------------------------------------------------------
# Launch 2: scoring
# --------------------------------------------------------------------------

def build_score_nc():
    nc = bacc.Bacc("TRN2", target_bir_lowering=False, debug=False,
                   num_devices=NCORES)

    hT6_d = nc.dram_tensor("hT6", [128, 6, 128], BF16, kind="ExternalInput")
    # three softmax-pre-weighted embedding tables (we_p = emb * tcw[:, p])
    weT_d = [nc.dram_tensor(f"weT{p}", [128, 2, T_PAD], BF16,
                            kind="ExternalInput") for p in range(3)]
    scores_d = nc.dram_tensor("scores", [128, T_PAD], BF16,
                              kind="ExternalOutput")

    with tile.TileContext(nc) as tc:
        with (
            tc.tile_pool(name="consts", bufs=1) as consts,
            tc.tile_pool(name="epsum", bufs=2, space="PSUM") as epsum,
            tc.tile_pool(name="outp", bufs=1) as outp,
        ):
            hT6 = consts.tile([128, 6, 128], BF16)
            nc.scalar.dma_start(hT6[:], hT6_d.ap())
            weT = [consts.tile([128, 2, T_PAD], BF16, name=f"weT{p}")
                   for p in range(3)]
            # interleaved slice-loads of the three tables: small first slice
            # so chunk 0 starts early, small last slice for a short drain
            QT = [(0, 4), (4, 12), (16, 13), (29, 13), (42, 7)]
            for qi, (q0, qn) in enumerate(QT):
                for p in range(3):
                    eng = nc.sync if (qi * 3 + p) % 2 == 0 else nc.scalar
                    eng.dma_start(weT[p][:, :, ds(q0 * 128, qn * 128)],
                                  weT_d[p].ap()[:, :, ds(q0 * 128, qn * 128)])

            # --- per-chunk scoring: PSUM-accumulate over purposes + k ---
            out_sb = outp.tile([128, T_PAD], BF16)
            for ci, (c0, cs) in enumerate(zip(CHUNK_OFFS[:-1], CHUNK_SIZES)):
                ps = epsum.tile([128, N_CHUNK], F32, tag=f"S{ci % 2}",
                                name="ps_s")
                for p in range(3):
                    for k in range(2):
                        nc.tensor.matmul(
                            ps[:, :cs], hT6[:, p * 2 + k, :],
                            weT[p][:, k, ds(c0, cs)],
                            start=(p == 0 and k == 0),
                            stop=(p == 2 and k == 1))
                nc.scalar.copy(out_sb[:, ds(c0, cs)], ps[:, :cs])
            # writeback batches shrink toward the end so the last DMAs fire
            # as soon as the final chunks land
            WB = [(0, 4), (4, 4), (8, 3), (11, 2)]
            for b0, bn in WB:
                o0 = CHUNK_OFFS[b0]
                o1 = CHUNK_OFFS[b0 + bn]
                nc.sync.dma_start(scores_d.ap()[:, ds(o0, o1 - o0)],
                                  out_sb[:, ds(o0, o1 - o0)])

    nc.compile()
    return nc


def score_host_inputs(hn_bf, emb, emb_purpose):
    # target concentration weights (input-only weight preprocessing, like
    # the emb[seq] gather): tcw = softmax(emb @ emb_purpose.T, axis=1),
    # folded into per-purpose pre-weighted tables we_p = emb * tcw[:, p]
    lg = emb @ emb_purpose.T                   # [T, 3]
    e = np.exp(lg - lg.max(axis=1, keepdims=True))
    tcw = (e / e.sum(axis=1, keepdims=True)).astype(np.float32)  # [T, 3]

    base = N_ITEMS // NCORES
    rem = N_ITEMS - base * NCORES
    bounds = []
    s0 = 0
    for c in range(NCORES):
        n = base + (1 if c < rem else 0)
        bounds.append((s0, s0 + n))
        s0 += n

    in_maps = []
    for c in range(NCORES):
        lo, hi = bounds[c]
        n = hi - lo
        m = {"hT6": hn_bf}
        for p in range(3):
            we = (emb[lo:hi] * tcw[lo:hi, p:p + 1]).T.astype(_BF)  # [256, n]
            weT = np.zeros((128, 2, T_PAD), _BF)
            weT[:, :, :n] = we.reshape(2, 128, n).transpose(1, 0, 2)
            m[f"weT{p}"] = weT
        in_maps.append(m)
    return in_maps, bounds


# --------------------------------------------------------------------------
# Entry point
# --------------------------------------------------------------------------

_SCAN_NC = None
_SCORE_NC = None


def _get_ncs():
    global _SCAN_NC, _SCORE_NC
    if _SCAN_NC is None:
        _SCAN_NC = build_scan_nc()
    if _SCORE_NC is None:
        _SCORE_NC = build_score_nc()
    return _SCAN_NC, _SCORE_NC


def kernel(seq, emb, emb_purpose, w_ih, w_hh, b_ih, b_hh):
    seq = np.asarray(seq)
    emb = np.asarray(emb, np.float32)
    emb_purpose = np.asarray(emb_purpose, np.float32)
    w_ih = np.asarray(w_ih, np.float32)
    w_hh = np.asarray(w_hh, np.float32)
    b_ih = np.asarray(b_ih, np.float32)
    b_hh = np.asarray(b_hh, np.float32)

    scan_nc, score_nc = _get_ncs()

    scan_ins = scan_host_inputs(seq, emb, emb_purpose, w_ih, w_hh, b_ih, b_hh)
    res1 = run_bass_kernel_spmd(scan_nc, scan_ins, core_ids=list(range(NCORES)))

    hT6 = np.zeros((128, 6, 128), _BF)
    for c in range(6):
        p, h = CORE_PH[c]
        sl = res1.results[c]["hn_out"].reshape(128, 2, BH)
        for k in range(2):
            hT6[:, p * 2 + k, h * BH:(h + 1) * BH] = sl[:, k, :]

    score_ins, bounds = score_host_inputs(hT6, emb, emb_purpose)
    res2 = run_bass_kernel_spmd(score_nc, score_ins, core_ids=list(range(NCORES)))

    scores = np.empty((B, N_ITEMS), np.float32)
    for c in range(NCORES):
        lo, hi = bounds[c]
        scores[:, lo:hi] = res2.results[c]["scores"][:, : hi - lo]\
            .astype(np.float32)
    return scores
